# revision 5
# baseline (speedup 1.0000x reference)
"""Multi-head attention (B=2, S=2048, D=1024, H=16) on 8 TRN2 NeuronCores.

Sharding: core c -> (batch b = c//4, head-group g = c%4 of 4 heads / 256 dims).
Per core: QKV projections for its head slice, attention for its 4 heads,
softmax normalization, AllGather of attention outputs across the 4 cores of
the batch group, then the core's 256-column slice of the output projection.
Host side only transposes/casts/slices inputs and concatenates outputs.

Layout notes:
- Activations are kept transposed ([feature, seq]) so every matmul contracts
  on the partition axis without on-chip transposes.
- Scores are computed transposed ([kseq, q]); softmax row sums come from 64
  ones-columns appended to each head of V, so the PV matmul emits the row sum
  replicated across partitions 64..127 and normalization is plain DVE math.
- No max-subtraction in softmax: scores are ~N(0,1) after the 1/sqrt(dk)
  scale, safely inside exp's fp32 range.

v2 performance structure (vs the v1 baseline):
- DMAs are consolidated into multi-dim access patterns (one DMA per weight
  tensor / per x chunk / per gather / per y chunk). Each DMA instruction
  costs ~650ns of serialized SP-sequencer + HWDGE time regardless of size,
  so 184 small DMAs cost ~120us of serial dispatch; ~36 large ones ~23us.
- The out-projection of chunk c is emitted one chunk late (after attention
  of chunk c+1) so its AllGather latency is hidden behind compute; the last
  chunk's out-projection is emitted inside the NEXT repeat's k/v projection
  phase (software pipeline across repeats).
- kTc/vE/x tiles are double-buffered so the next repeat's projections can
  run while this repeat's tail drains.
"""

import numpy as np
import ml_dtypes

import concourse.bass as bass
import concourse.mybir as mybir
import concourse.tile as tile
from concourse.bass_utils import run_bass_kernel_spmd

BF16 = ml_dtypes.bfloat16
F32 = mybir.dt.float32
BF = mybir.dt.bfloat16

B, S, D, H = 2, 2048, 1024, 16
DK = D // H          # 64
HPC = H // 4         # 4 heads per core
EG = D // 4          # 256 dims per head-group
KT = D // 128        # 8 contraction tiles
GROUPS = [[0, 1, 2, 3], [4, 5, 6, 7]]
EXP = mybir.ActivationFunctionType.Exp

TRACE = False
LAST_EXEC_NS = None


# --- workaround: this walrus build only encodes ONE sync wait per
# instruction ("Too many sync wait commands" in setupSyncWait). Hoist
# excess waits onto same-engine NOP carriers placed just before the
# instruction; engines execute in order, so semantics are unchanged. ---
def _split_multi_waits(nc, max_waits=1):
    n = 0
    for f in nc.m.functions:
        for bb in f.blocks:
            new = []
            for inst in bb.instructions:
                si = inst.sync_info
                waits = list(si.on_wait) if si is not None and si.on_wait else []
                if len(waits) > max_waits:
                    keep = len(waits) - max_waits
                    for j in range(0, keep, max_waits):
                        n += 1
                        new.append(
                            mybir.InstNoOp(
                                name=f"waitsplit-{n}",
                                engine=inst.engine,
                                bass_nofuse=True,
                                sync_info=mybir.SyncInfo(
                                    on_wait=waits[j : j + max_waits], on_update=[]
                                ),
                            )
                        )
                    si.on_wait = waits[keep:]
                new.append(inst)
            bb.instructions[:] = new
    return n


def build(s=S, repeat=1, defer=True, agchunk=False):
    """Build the per-core SPMD program. s = sequence length (tunable for sim).
    repeat > 1 re-runs the whole computation for wall-clock benchmarking.

    defer: emit out-projection of chunk c after attention of chunk c+1
    (and the last chunk's inside the next repeat's projection phase).
    agchunk: one AllGather per chunk ([256,512]->[1024,512]) instead of
    one per (chunk, head-pair) ([128,512]->[512,512]).
    """
    n_sc = s // 512   # 512-wide q chunks
    n_st = s // 128   # 128-wide seq tiles

    nc = bass.Bass(num_devices=8)
    xq_t = nc.declare_dram_parameter("xq_t", [D, s], BF, isOutput=False)
    xk_t = nc.declare_dram_parameter("xk_t", [D, s], BF, isOutput=False)
    xv_t = nc.declare_dram_parameter("xv_t", [D, s], BF, isOutput=False)
    wq_t = nc.declare_dram_parameter("wq_t", [D, EG], BF, isOutput=False)
    wk_t = nc.declare_dram_parameter("wk_t", [D, EG], BF, isOutput=False)
    wv_t = nc.declare_dram_parameter("wv_t", [D, EG], BF, isOutput=False)
    wo_t = nc.declare_dram_parameter("wo_t", [D, EG], BF, isOutput=False)
    y_ext = nc.declare_dram_parameter("y", [s, EG], F32, isOutput=True)

    if agchunk:
        bounce = [nc.dram_tensor(f"attn_bounce{c}", [EG, 512], BF)
                  for c in range(n_sc)]
        gath = [nc.dram_tensor(f"attn_gath{c}", [D, 512], BF)
                for c in range(n_sc)]
    else:
        bounce = [[nc.dram_tensor(f"attn_bounce{c}_{p}", [128, 512], BF)
                   for p in range(2)] for c in range(n_sc)]
        gath = [[nc.dram_tensor(f"attn_gath{c}_{p}", [512, 512], BF)
                 for p in range(2)] for c in range(n_sc)]

    with tile.TileContext(nc) as tc:
        with (
            tc.tile_pool(name="kvp", bufs=2) as kvp,
            tc.tile_pool(name="wpool", bufs=1) as wp,
            tc.tile_pool(name="xpool", bufs=2) as xp,
            tc.tile_pool(name="psum2", bufs=1, space="PSUM") as ps2,
            tc.tile_pool(name="expp", bufs=3) as ep,
            tc.tile_pool(name="normp", bufs=2) as np_,
            tc.tile_pool(name="qcp", bufs=2) as qcp,
            tc.tile_pool(name="acp", bufs=2) as acp,
            tc.tile_pool(name="agp", bufs=2) as agp,
            tc.tile_pool(name="yp", bufs=2) as yp,
        ):
            # weights: one [128, KT, EG] tile per tensor, one DMA each.
            wts = {}
            for nm, src in (("wq", wq_t), ("wk", wk_t), ("wv", wv_t),
                            ("wo", wo_t)):
                wts[nm] = wp.tile([128, KT, EG], BF, tag=nm, name=nm)
                nc.sync.dma_start(
                    wts[nm][:], src[:].rearrange("(k p) c -> p k c", k=KT)
                )
            wq, wk, wv, wo_sb = wts["wq"], wts["wk"], wts["wv"], wts["wo"]

            def emit_outproj(c):
                """Gather-read + out-projection + y write for chunk c."""
                agT = [agp.tile([128, 4, 512], BF, tag=f"agT{p}",
                                name=f"agT{c}_{p}") for p in range(2)]
                for p in range(2):
                    src = (gath[c][p * 512:(p + 1) * 512, :] if agchunk
                           else gath[c][p][:])
                    nc.sync.dma_start(
                        agT[p][:], src.rearrange("(r p) c -> p r c", r=4)
                    )
                if agchunk:
                    # gath rows are k-major: agT[p][:, r] holds block 4p+r
                    korder = [(k // 4, k % 4, k) for k in range(KT)]
                else:
                    # block k = 2r+p; pair-0 gather lands half a chunk
                    # earlier, use it first so accumulation overlaps pair-1
                    korder = [(p, r, 2 * r + p) for p in range(2)
                              for r in range(4)]
                ysb = yp.tile([128, 4, EG], F32, tag="ysb", name=f"ysb{c}")
                for qt in range(4):
                    yps = ps2.tile([128, EG], F32, tag="p1", bufs=2,
                                   name=f"yps{c}_{qt}")
                    for i, (p, r, k) in enumerate(korder):
                        nc.tensor.matmul(
                            yps[:],
                            agT[p][:, r, qt * 128:(qt + 1) * 128],
                            wo_sb[:, k, :],
                            start=(i == 0),
                            stop=(i == KT - 1),
                        )
                    nc.vector.tensor_copy(ysb[:, qt, :], yps[:])
                nc.sync.dma_start(
                    y_ext[c * 512:(c + 1) * 512, :]
                    .rearrange("(qt p) c -> p qt c", qt=4),
                    ysb[:],
                )

            have_tail = False  # deferred chunk-3 out-proj pending?
            for _rep in range(repeat):
                # ---------- phase 1: k/v projections, chunked along s ------
                kTc = [[None] * n_sc, [None] * n_sc]
                vE = [None] * n_st
                for c2 in range(n_sc):
                    cs2 = slice(c2 * 512, (c2 + 1) * 512)
                    xk = xp.tile([128, KT, 512], BF, tag="xk", name=f"xk{_rep}_{c2}")
                    xv = xp.tile([128, KT, 512], BF, tag="xv", name=f"xv{_rep}_{c2}")
                    nc.sync.dma_start(
                        xk[:], xk_t[:, cs2].rearrange("(k p) c -> p k c", k=KT)
                    )
                    nc.sync.dma_start(
                        xv[:], xv_t[:, cs2].rearrange("(k p) c -> p k c", k=KT)
                    )
                    for e in range(2):
                        ps = ps2.tile([128, 512], F32, tag="p1", bufs=2,
                                      name=f"pk{_rep}_{c2}{e}")
                        for k in range(KT):
                            nc.tensor.matmul(
                                ps[:],
                                wk[:, k, e * 128:(e + 1) * 128],
                                xk[:, k, :],
                                start=(k == 0),
                                stop=(k == KT - 1),
                            )
                        kTc[e][c2] = kvp.tile([128, 512], BF, tag=f"kTc{e}_{c2}",
                                              name=f"kTc{_rep}_{e}_{c2}")
                        nc.vector.tensor_copy(kTc[e][c2][:], ps[:])
                    for t in range(4 * c2, 4 * c2 + 4):
                        tl = slice((t % 4) * 128, (t % 4) * 128 + 128)
                        ps = ps2.tile([128, EG], F32, tag="p1", bufs=2,
                                      name=f"pv{_rep}_{t}")
                        for k in range(KT):
                            nc.tensor.matmul(
                                ps[:],
                                xv[:, k, tl],
                                wv[:, k, :],
                                start=(k == 0),
                                stop=(k == KT - 1),
                            )
                        vE[t] = kvp.tile([128, HPC * 2 * DK], BF, tag=f"vE{t}",
                                         name=f"vE{_rep}_{t}")
                        nc.vector.memset(vE[t][:], 1.0)
                        for h in range(HPC):
                            nc.vector.tensor_copy(
                                vE[t][:, h * 2 * DK:h * 2 * DK + DK],
                                ps[:, h * DK:(h + 1) * DK],
                            )
                    if c2 == 1 and have_tail:
                        # previous repeat's chunk-3 out-projection: its
                        # AllGather has had two projection chunks to land.
                        emit_outproj(n_sc - 1)
                        have_tail = False

                # ---------- phase 2+3: per-chunk attention pipeline --------
                for c in range(n_sc):
                    cs = slice(c * 512, (c + 1) * 512)
                    xq = xp.tile([128, KT, 512], BF, tag="xq", name=f"xq{_rep}_{c}")
                    nc.sync.dma_start(
                        xq[:], xq_t[:, cs].rearrange("(k p) c -> p k c", k=KT)
                    )
                    # q projection for this chunk (borrows a scores slot)
                    qp = ps2.tile([128, 1024], F32, tag="scores", bufs=2,
                                  name=f"qp{_rep}_{c}")
                    for e in range(2):
                        for k in range(KT):
                            nc.tensor.matmul(
                                qp[:, e * 512:(e + 1) * 512],
                                wq[:, k, e * 128:(e + 1) * 128],
                                xq[:, k, :],
                                start=(k == 0),
                                stop=(k == KT - 1),
                            )
                    qTc = [qcp.tile([128, 512], BF, tag=f"qTc{e}",
                                    name=f"qTc{_rep}_{c}_{e}") for e in range(2)]
                    for e in range(2):
                        nc.vector.tensor_copy(qTc[e][:], qp[:, e * 512:(e + 1) * 512])

                    attnc = [acp.tile([128, 512], BF, tag=f"attnc{t2}",
                                      name=f"attnc{_rep}_{c}_{t2}") for t2 in range(2)]
                    for pair in range(2):
                        aP = [ps2.tile([128, 512], F32, tag=f"attnP{sub}",
                                       name=f"aP{_rep}_{c}_{pair}_{sub}")
                              for sub in range(2)]
                        for t in range(n_st):
                            scp = ps2.tile([128, 1024], F32, tag="scores", bufs=2,
                                           name=f"sc{_rep}_{c}_{pair}_{t}")
                            for sub in range(2):
                                row = slice(64 * sub, 64 * sub + 64)
                                nc.tensor.matmul(
                                    scp[:, sub * 512:(sub + 1) * 512],
                                    kTc[pair][t // 4][row, (t % 4) * 128:(t % 4) * 128 + 128],
                                    qTc[pair][row, :],
                                    start=True,
                                    stop=True,
                                )
                            ex = ep.tile([128, 1024], BF, tag="expT", bufs=3,
                                         name=f"ex{_rep}_{c}_{pair}_{t}")
                            nc.scalar.activation(ex[:], scp[:], EXP, scale=1.0 / 8.0)
                            for sub in range(2):
                                h = 2 * pair + sub
                                nc.tensor.matmul(
                                    aP[sub][:],
                                    vE[t][:, h * 2 * DK:(h + 1) * 2 * DK],
                                    ex[:, sub * 512:(sub + 1) * 512],
                                    start=(t == 0),
                                    stop=(t == n_st - 1),
                                )
                        for sub in range(2):
                            den = np_.tile([DK, 512], F32, tag="den",
                                           name=f"den{_rep}_{c}_{pair}_{sub}")
                            nc.vector.reciprocal(den[:], aP[sub][DK:2 * DK, :])
                            nc.vector.tensor_mul(
                                attnc[pair][64 * sub:64 * sub + 64, :],
                                aP[sub][0:DK, :],
                                den[:],
                            )
                        if agchunk:
                            nc.sync.dma_start(
                                bounce[c][pair * 128:(pair + 1) * 128, :],
                                attnc[pair][:],
                            )
                        else:
                            nc.sync.dma_start(bounce[c][pair][:], attnc[pair][:])
                            nc.gpsimd.collective_compute(
                                "AllGather",
                                mybir.AluOpType.bypass,
                                replica_groups=GROUPS,
                                ins=[bounce[c][pair][:]],
                                outs=[gath[c][pair][:]],
                            )
                    if agchunk:
                        nc.gpsimd.collective_compute(
                            "AllGather",
                            mybir.AluOpType.bypass,
                            replica_groups=GROUPS,
                            ins=[bounce[c][:]],
                            outs=[gath[c][:]],
                        )
                    if not defer:
                        emit_outproj(c)
                    elif c >= 1:
                        emit_outproj(c - 1)
                if defer:
                    have_tail = True
            if have_tail:
                emit_outproj(n_sc - 1)

    _split_multi_waits(nc)
    return nc


def _bf16_c(a):
    return np.ascontiguousarray(a).astype(BF16)


def kernel(query, key, value, Wq, bq, Wk, bk, Wv, bv, Wo, bo):
    global LAST_EXEC_NS
    query, key, value = (np.asarray(a, np.float32) for a in (query, key, value))
    Wq, Wk, Wv, Wo = (np.asarray(a, np.float32) for a in (Wq, Wk, Wv, Wo))
    for b_ in (bq, bk, bv, bo):
        assert not np.any(np.asarray(b_)), "nonzero biases not supported"

    nc = build(S)
    in_maps = []
    for c in range(8):
        b, g = divmod(c, 4)
        eg = slice(EG * g, EG * (g + 1))
        in_maps.append(
            {
                "xq_t": _bf16_c(query[b].T),
                "xk_t": _bf16_c(key[b].T),
                "xv_t": _bf16_c(value[b].T),
                "wq_t": _bf16_c(Wq[eg].T),
                "wk_t": _bf16_c(Wk[eg].T),
                "wv_t": _bf16_c(Wv[eg].T),
                "wo_t": _bf16_c(Wo[eg].T),
            }
        )
    res = run_bass_kernel_spmd(nc, in_maps, list(range(8)), trace=TRACE)
    LAST_EXEC_NS = res.exec_time_ns
    y = np.empty((B, S, D), np.float32)
    for c in range(8):
        b, g = divmod(c, 4)
        y[b][:, EG * g:EG * (g + 1)] = res.results[c]["y"]
    return y
